# revision 16
# baseline (speedup 1.0000x reference)
"""GATv2 convolution on 8 Trainium2 NeuronCores (Bass/Tile) — v4.

Algorithm (identical numerics to v2/v3):
  - Host: edges sharded by tgt//nslice; per 128-node target tile, edges
    bucketed by src block (src//128) into uniform W=32 slot segments so
    one SPMD program fits all cores (pads srclo/tgtloc = -1).
  - Phase A: T1a = x @ (w1*|a|) resident in SBUF [128, 392, 128] bf16;
    T2a = xs @ (w2*|a|); T2f = T2a/|a| (f32). For_i over block groups.
  - Phase B: doubly-nested For_i (49 tiles x 14 groups of 7 chunks).
    Per group: one-hot builds (is_equal), z = T1a[src] + T2a[tgt] via
    PSUM matmuls (1 tgt + 4 W=32-offset segment matmuls per chunk),
    y = Prelu(z) (ACT), ew = y*sign(a), logits reduce, w = exp (ACT),
    scat = [w*z | w], scatter matmul into per-group PSUM acc, then
    SBUF accumulator += acc. out = accS_num/den * inv|a| - T2f.
  - Static instruction count is minimized via hardware loops: measured
    wall cost on this target is ~0.1-0.2 ms per STATIC instruction
    (program load), execution itself is a few ms.
"""

import os
import sys

sys.path.insert(0, "/opt/trn_rl_repo")

import numpy as np
import ml_dtypes

import concourse.bass as bass
import concourse.bacc as bacc
import concourse.mybir as mybir
import concourse.tile as tile
from concourse import bass_utils
from concourse.bass import ds
from concourse.masks import make_identity

P = 128
CORES = 8
ALPHA = 0.3
NH = 8
OC = 16
W = 32           # slot segment width per (tile, src-block)
NSEG = 392       # src blocks incl. zero-pad block (50176 / 128)
SPC = P // W     # segments per 128-slot chunk
G = 7            # chunks per group (98 = 14 * 7)
NGR = 14         # groups per tile

f32 = mybir.dt.float32
bf16 = mybir.dt.bfloat16

_last_results = None  # test harness reads exec_time_ns from here


def _td_np(td):
    return ml_dtypes.bfloat16 if td == bf16 else np.float32


def _host_prep(x, w1, w2, a, src, tgt, td_np=None):
    N, CH = x.shape
    E = src.shape[0]
    assert CH == P and N % CORES == 0
    nslice = N // CORES
    nt_b = (nslice + P - 1) // P
    n_pad = NSEG * P
    xsr = nt_b * P
    ts = NSEG * W
    t_c = ts // P
    assert t_c == NGR * G

    src = src.astype(np.int64)
    tgt = tgt.astype(np.int64)
    deg = np.bincount(tgt, minlength=N)
    assert deg.min() >= 1, "empty target nodes unsupported (out-T2 formula)"

    core = tgt // nslice
    tloc = tgt - core * nslice
    tile_i = tloc // P
    loc = tloc % P
    # x table is the AllGather of per-core padded shards: node n of core c
    # lives at row c*xsr + (n - c*nslice)
    src_p = src + (src // nslice) * (xsr - nslice)
    seg = src_p // P
    row = src_p % P

    key = (core * nt_b + tile_i) * NSEG + seg
    order = np.argsort(key, kind="stable")
    key_s = key[order]
    counts = np.bincount(key, minlength=CORES * nt_b * NSEG)
    assert counts.max() <= W, f"segment overflow: {counts.max()} > {W}"
    gstart = np.zeros_like(counts)
    gstart[1:] = np.cumsum(counts)[:-1]
    rank = np.arange(E, dtype=np.int64) - gstart[key_s]
    slot = (key_s % NSEG) * W + rank
    ct = key_s // NSEG

    srclo_arr = np.full((CORES, nt_b, ts), -1.0, dtype=np.float32)
    tgl_arr = np.full((CORES, nt_b, ts), -1.0, dtype=np.float32)
    srclo_arr[ct // nt_b, ct % nt_b, slot] = row[order]
    tgl_arr[ct // nt_b, ct % nt_b, slot] = loc[order]

    # group-major layouts: [nt_b*NGR, ...] so one dynamic index suffices
    srclo_r = srclo_arr.reshape(CORES, nt_b * NGR, 1, G * P).astype(
        ml_dtypes.bfloat16
    )
    tglr_r = tgl_arr.reshape(CORES, nt_b * NGR, 1, G * P).astype(
        ml_dtypes.bfloat16
    )
    tg = np.ascontiguousarray(
        tgl_arr.reshape(CORES, nt_b * NGR, G, P).transpose(0, 1, 3, 2)
    ).astype(ml_dtypes.bfloat16)

    xbf = x.astype(ml_dtypes.bfloat16)
    xs_pad = np.zeros((CORES, xsr, CH), dtype=ml_dtypes.bfloat16)
    for c in range(CORES):
        xs_pad[c, :nslice] = xbf[c * nslice : (c + 1) * nslice]

    a1 = a.reshape(CH).astype(np.float64)
    aabs = np.maximum(np.abs(a1), 1e-6)
    asgn = np.where(a1 >= 0, 1.0, -1.0)
    w12a = np.concatenate(
        [w1 * aabs[None, :], w2 * aabs[None, :]], axis=1
    ).astype(np.float32)
    sbc = np.tile(asgn[None, :], (P, 1)).astype(ml_dtypes.bfloat16)
    invabc = np.tile((1.0 / aabs)[None, :], (P, 1)).astype(np.float32)
    iop = np.tile(np.arange(P, dtype=np.float32)[:, None], (1, P)).astype(
        ml_dtypes.bfloat16
    )
    iota = np.tile(np.arange(P, dtype=np.float32)[None, :], (P, 1)).astype(
        ml_dtypes.bfloat16
    )

    in_maps = []
    for c in range(CORES):
        in_maps.append(
            {
                "xs": np.ascontiguousarray(xs_pad[c]),
                "w12a": w12a,
                "sbc": sbc,
                "invabc": invabc,
                "iop": iop,
                "srclo": np.ascontiguousarray(srclo_r[c]),
                "tglr": np.ascontiguousarray(tglr_r[c]),
            }
        )
    dims = dict(
        N=N, CH=CH, nslice=nslice, nt_b=nt_b, n_pad=n_pad, xsr=xsr,
        ts=ts, t_c=t_c,
    )
    return in_maps, dims


def _build_program(dims, td=bf16):
    CH = dims["CH"]
    nt_b = dims["nt_b"]
    n_pad = dims["n_pad"]
    xsr = dims["xsr"]
    ts = dims["ts"]
    kstage = os.environ.get("KSTAGE", "full")
    GA = int(os.environ.get("KGA", "7"))     # blocks per phase-A group
    assert NSEG % GA == 0 and nt_b % GA == 0

    nc = bacc.Bacc("TRN2", target_bir_lowering=False, debug=False,
                   num_devices=CORES)

    xs_in = nc.dram_tensor("xs", [xsr, CH], bf16, kind="ExternalInput")
    # collectives aren't supported on I/O tensors -> bounce + gather bufs
    xsb = nc.dram_tensor("xsb", [xsr, CH], bf16, kind="Internal")
    xg = nc.dram_tensor("xg", [n_pad, CH], bf16, kind="Internal",
                        addr_space="Shared")
    w12a_in = nc.dram_tensor("w12a", [CH, 2 * CH], f32, kind="ExternalInput")
    sbc_in = nc.dram_tensor("sbc", [P, CH], bf16, kind="ExternalInput")
    inva_in = nc.dram_tensor("invabc", [P, CH], f32, kind="ExternalInput")
    iop_in = nc.dram_tensor("iop", [P, P], bf16, kind="ExternalInput")
    srclo_in = nc.dram_tensor("srclo", [nt_b * NGR, 1, G * P], bf16,
                              kind="ExternalInput")
    tglr_in = nc.dram_tensor("tglr", [nt_b * NGR, 1, G * P], bf16,
                             kind="ExternalInput")
    # padded output: uniform 128 rows per tile; host slices [:nslice]
    out = nc.dram_tensor("out", [nt_b * P, CH], f32, kind="ExternalOutput")
    out_r = out[:].rearrange("(t p) c -> t p c", p=P)

    with tile.TileContext(nc) as tc:
        with tc.tile_pool(name="const", bufs=1) as cp:
            ident = cp.tile([P, P], f32)
            make_identity(nc, ident[:])
            identb = cp.tile([P, P], bf16)
            nc.vector.tensor_copy(out=identb[:], in_=ident[:])
            w12f = cp.tile([CH, 2 * CH], f32)
            nc.sync.dma_start(out=w12f[:], in_=w12a_in[:])
            w12t = cp.tile([CH, 2 * CH], bf16)
            nc.vector.tensor_copy(out=w12t[:], in_=w12f[:])
            sbc_t = cp.tile([P, CH], bf16)
            nc.sync.dma_start(out=sbc_t[:], in_=sbc_in[:])
            inva_t = cp.tile([P, CH], f32)
            nc.sync.dma_start(out=inva_t[:], in_=inva_in[:])
            iop_t = cp.tile([P, P], bf16)
            nc.sync.dma_start(out=iop_t[:], in_=iop_in[:])

            t1sb = cp.tile([P, NSEG, CH], bf16, tag="t1sb")
            t2a_sb = cp.tile([P, nt_b, CH], bf16, tag="t2asb")
            t2f_sb = cp.tile([P, nt_b, CH], f32, tag="t2fsb")
            # t1sb view for dynamic group indexing: seg = gi*(G*SPC) + s
            t1_g = t1sb[:].rearrange("p (gi s) c -> p gi s c", s=G * SPC)

            # ---------------- Phase A: projection tables (SBUF) ----------
            if kstage != "IO":
                nc.sync.dma_start(out=xsb[:], in_=xs_in[:])
                nc.gpsimd.collective_compute(
                    "AllGather",
                    mybir.AluOpType.bypass,
                    replica_groups=[list(range(CORES))],
                    ins=[xsb[:]],
                    outs=[xg[:]],
                )
            x_v = xg[:].rearrange("(b r) c -> b r c", r=GA * P)
            t1_v = t1sb[:].rearrange("p (b k) c -> p b k c", k=GA)
            xs_v = xs_in[:].rearrange("(b r) c -> b r c", r=GA * P)
            t2a_v = t2a_sb[:].rearrange("p (b k) c -> p b k c", k=GA)
            t2f_v = t2f_sb[:].rearrange("p (b k) c -> p b k c", k=GA)

            with (
                tc.tile_pool(name="pa", bufs=2) as pa,
                tc.tile_pool(name="pa_ps", bufs=2, space="PSUM") as pa_ps,
                tc.tile_pool(name="pa_ps2", bufs=2, space="PSUM") as pa_ps2,
            ):
                def project_group(src_v, b, w_sl, dst_v, t2fd=None):
                    xt4 = pa.tile([P, GA, CH], bf16, tag="xt")
                    nc.sync.dma_start(
                        out=xt4[:],
                        in_=src_v[ds(b, 1)].rearrange(
                            "o (k p) c -> p (o k) c", p=P
                        ),
                    )
                    psT = pa_ps.tile([P, GA * P], bf16, space="PSUM",
                                     tag="psT")
                    for k in range(GA):
                        nc.tensor.transpose(
                            out=psT[:, k * P : (k + 1) * P],
                            in_=xt4[:, k, :],
                            identity=identb[:],
                        )
                    xT = pa.tile([P, GA * P], bf16, tag="xT")
                    nc.vector.tensor_copy(out=xT[:], in_=psT[:])
                    mm = pa_ps2.tile([P, GA * CH], f32, space="PSUM",
                                     tag="mm")
                    for k in range(GA):
                        nc.tensor.matmul(
                            out=mm[:, k * CH : (k + 1) * CH],
                            lhsT=xT[:, k * P : (k + 1) * P],
                            rhs=w_sl,
                            start=True,
                            stop=True,
                        )
                    nc.scalar.copy(
                        out=dst_v[:, ds(b, 1), :, :].rearrange(
                            "p o k c -> p (o k c)"
                        ),
                        in_=mm[:],
                    )
                    if t2fd is not None:
                        nc.vector.tensor_tensor(
                            out=t2fd[:, ds(b, 1), :, :].rearrange(
                                "p o k c -> p k (o c)"
                            ),
                            in0=mm[:].rearrange("p (k c) -> p k c", c=CH),
                            in1=inva_t[:][:, None, :].broadcast_to(
                                [P, GA, CH]
                            ),
                            op=mybir.AluOpType.mult,
                        )

                if kstage != "IO":
                    with tc.For_i(0, NSEG // GA) as b:
                        project_group(x_v, b, w12t[:, 0:CH], t1_v)
                    with tc.For_i(0, nt_b // GA) as b2:
                        project_group(xs_v, b2, w12t[:, CH : 2 * CH],
                                      t2a_v, t2fd=t2f_v)

            # ---------------- Phase B: edge processing -------------------
            with (
                tc.tile_pool(name="pb", bufs=2) as pb,
                tc.tile_pool(name="pbs", bufs=1) as pbs,
                tc.tile_pool(name="pbg", bufs=2) as pbg,
                tc.tile_pool(name="pb_ps", bufs=2, space="PSUM") as pb_ps,
                tc.tile_pool(name="pb_acc", bufs=2, space="PSUM") as pb_acc,
            ):
                if kstage == "IO":
                    zt0 = pb.tile([P, CH], f32, tag="ot")
                    nc.vector.tensor_copy(out=zt0[:], in_=ident[:])
                    nc.sync.dma_start(out=out[0:P, :], in_=zt0[:])
                else:
                    cols = G * P
                    with tc.For_i(0, nt_b) as t:
                        accS = pbs.tile([P, 136], f32, tag="accS")
                        nc.vector.memset(accS[:], 0.0)
                        with tc.For_i(0, NGR) as gi:
                            idx = t * NGR + gi
                            srcb = pbg.tile([P, G * P], bf16, tag="srcb")
                            nc.sync.dma_start(
                                out=srcb[:],
                                in_=srclo_in[ds(idx, 1), 0:1, :]
                                .rearrange("o q f -> (o q) f")
                                .broadcast_to([P, cols]),
                            )
                            tgb = pbg.tile([P, G * P], bf16, tag="tgb")
                            nc.scalar.dma_start(
                                out=tgb[:],
                                in_=tglr_in[ds(idx, 1), 0:1, :]
                                .rearrange("o q f -> (o q) f")
                                .broadcast_to([P, cols]),
                            )
                            ohs = pbg.tile([P, G * P], bf16, tag="ohs")
                            nc.vector.tensor_tensor(
                                out=ohs[:],
                                in0=iop_t[:][:, 0:1].broadcast_to([P, cols]),
                                in1=srcb[:],
                                op=mybir.AluOpType.is_equal,
                            )
                            oht = pbg.tile([P, G * P], bf16, tag="oht")
                            nc.vector.tensor_tensor(
                                out=oht[:],
                                in0=iop_t[:][:, 0:1].broadcast_to([P, cols]),
                                in1=tgb[:],
                                op=mybir.AluOpType.is_equal,
                            )
                            psO = pb_ps.tile([P, G * P], bf16,
                                             space="PSUM", tag="psO")
                            for ck in range(G):
                                nc.tensor.transpose(
                                    out=psO[:, ck * P : (ck + 1) * P],
                                    in_=oht[:, ck * P : (ck + 1) * P],
                                    identity=identb[:],
                                )
                            oh = pbg.tile([P, G * P], bf16, tag="oh")
                            nc.scalar.copy(out=oh[:], in_=psO[:])

                            zps = pb_ps.tile([P, G * P], f32, space="PSUM",
                                             tag="zps")
                            t2a_sl = t2a_sb[:, ds(t, 1), :].rearrange(
                                "p o c -> p (o c)"
                            )
                            for ck in range(G):
                                cc = ck * P
                                nc.tensor.matmul(
                                    out=zps[:, cc : cc + P],
                                    lhsT=oht[:, cc : cc + P],
                                    rhs=t2a_sl,
                                    start=True,
                                    stop=False,
                                )
                                for j in range(SPC):
                                    s = ck * SPC + j
                                    nc.tensor.matmul(
                                        out=zps[
                                            j * W : (j + 1) * W, cc : cc + P
                                        ],
                                        lhsT=ohs[
                                            :, cc + j * W : cc + (j + 1) * W
                                        ],
                                        rhs=t1_g[:, ds(gi, 1), s, :]
                                        .rearrange("p o c -> p (o c)"),
                                        start=False,
                                        stop=(j == SPC - 1),
                                        tile_position=(0, j * W),
                                    )

                            y = pb.tile([P, G * P], bf16, tag="y")
                            nc.scalar.activation(
                                out=y[:], in_=zps[:],
                                func=mybir.ActivationFunctionType.Prelu,
                                alpha=ALPHA,
                            )
                            nc.vector.tensor_tensor(
                                out=y[:].rearrange("p (k c) -> p k c", c=CH),
                                in0=y[:].rearrange("p (k c) -> p k c", c=CH),
                                in1=sbc_t[:][:, None, :].broadcast_to(
                                    [P, G, CH]
                                ),
                                op=mybir.AluOpType.mult,
                            )
                            lg = pb.tile([P, G * NH], f32, tag="lg")
                            nc.vector.tensor_reduce(
                                out=lg[:].rearrange("p (k h) -> p k h", h=NH),
                                in_=y[:].rearrange(
                                    "p (k h c) -> p k h c", h=NH, c=OC
                                ),
                                axis=mybir.AxisListType.X,
                                op=mybir.AluOpType.add,
                            )
                            scat = pb.tile([P, G * 136], bf16, tag="scat")
                            scat_r = scat[:].rearrange(
                                "p (k c) -> p k c", c=136
                            )
                            nc.scalar.activation(
                                out=scat_r[:, :, CH : CH + NH],
                                in_=lg[:].rearrange("p (k h) -> p k h", h=NH),
                                func=mybir.ActivationFunctionType.Exp,
                            )
                            w_bc = scat_r[:, :, CH : CH + NH][
                                :, :, :, None
                            ].broadcast_to([P, G, NH, OC])
                            nc.vector.tensor_tensor(
                                out=scat_r[:, :, 0:CH].rearrange(
                                    "p k (h c) -> p k h c", c=OC
                                ),
                                in0=zps[:].rearrange(
                                    "p (k h c) -> p k h c", h=NH, c=OC
                                ),
                                in1=w_bc,
                                op=mybir.AluOpType.mult,
                            )

                            acc = pb_acc.tile([P, 136], f32, space="PSUM",
                                              tag="acc")
                            for ck in range(G):
                                nc.tensor.matmul(
                                    out=acc[:],
                                    lhsT=oh[:, ck * P : (ck + 1) * P],
                                    rhs=scat[:, ck * 136 : ck * 136 + 136],
                                    start=(ck == 0),
                                    stop=(ck == G - 1),
                                )
                            nc.vector.tensor_tensor(
                                out=accS[:], in0=accS[:], in1=acc[:],
                                op=mybir.AluOpType.add,
                            )

                        dg = pb.tile([P, NH], f32, tag="dg")
                        nc.vector.tensor_scalar_max(
                            out=dg[:], in0=accS[:, CH : CH + NH],
                            scalar1=1e-30,
                        )
                        rc = pb.tile([P, NH], f32, tag="rc")
                        nc.vector.reciprocal(out=rc[:], in_=dg[:])
                        ot = pb.tile([P, CH], f32, tag="ot")
                        nc.vector.tensor_tensor(
                            out=ot[:].rearrange("p (h c) -> p h c", c=OC),
                            in0=accS[:, 0:CH].rearrange(
                                "p (h c) -> p h c", c=OC
                            ),
                            in1=rc[:][:, :, None].broadcast_to([P, NH, OC]),
                            op=mybir.AluOpType.mult,
                        )
                        nc.vector.tensor_tensor(
                            out=ot[:], in0=ot[:], in1=inva_t[:],
                            op=mybir.AluOpType.mult,
                        )
                        nc.vector.tensor_tensor(
                            out=ot[:],
                            in0=ot[:],
                            in1=t2f_sb[:, ds(t, 1), :].rearrange(
                                "p o c -> p (o c)"
                            ),
                            op=mybir.AluOpType.subtract,
                        )
                        nc.sync.dma_start(
                            out=out_r[ds(t, 1), :, :].rearrange(
                                "o p c -> (o p) c"
                            ),
                            in_=ot[:],
                        )

    nc.compile()
    return nc


def kernel(x, w1, w2, a, src, tgt):
    global _last_results
    x = np.asarray(x, dtype=np.float32)
    w1 = np.asarray(w1, dtype=np.float32)
    w2 = np.asarray(w2, dtype=np.float32)
    a = np.asarray(a, dtype=np.float32)
    src = np.asarray(src)
    tgt = np.asarray(tgt)

    in_maps, dims = _host_prep(x, w1, w2, a, src, tgt)
    nc = _build_program(dims)

    trace = bool(os.environ.get("KBENCH_TRACE"))
    res = bass_utils.run_bass_kernel_spmd(
        nc, in_maps, core_ids=list(range(CORES)), trace=trace
    )
    _last_results = res
    nslice = dims["nslice"]
    out = np.empty((x.shape[0], x.shape[1]), dtype=np.float32)
    for c in range(CORES):
        out[c * nslice : (c + 1) * nslice] = res.results[c]["out"][:nslice]
    return out
